# revision 9
# baseline (speedup 1.0000x reference)
"""Trainium2 Bass kernel for nn_Adaptive_Pooling_Layer (B=16, N=2048, D=256, H=8, M=256).

Data-parallel over batch: 8 NeuronCores x 2 batches each; params replicated.

Math notes
----------
The module's input2centroids layer has weight/bias == 0 (torch zeros init), so
x = relu(bc @ 0 + 0) = 0 and centroids = relu(lin_b) broadcast over (batch, d):
centroids[b,h,m,d] = r[h,m] := relu(lin_b[h*M+m])  (constant in b and d).
Hence c_n[h,m,d] = cval[h,m] := r / max(16*r, 1e-6)   (sqrt(D)=16), and with
  rs[n]  = sum_d ns[n,d],   S[h] = sum_m cval[h,m]
  g[n,h] = t/(S[h]*t + 1e-10)  with t = rs/||ns[n,:]||
         = rs[n] / (S[h]*rs[n] + 1e-10*||ns[n,:]||)
        ~= rs[n] / (S[h]*rs[n] + 1.6e-9)          (||row|| = 16 +- 0.7 whp;
  the 1e-10 guard only matters at |rs| ~ 1e-9 which randn inputs never hit,
  so the row norm - and with it the whole sqrt/square pipeline - drops out)
the normalized C_heads[b,h,m,n] = cval[h,m] * g[b,n,h], so C = A_aug @ g_aug^T
with A_aug = [conv_w*cval^T | conv_b*1]  (M x 9).  Then
  new_node_set = A_aug @ (g_aug^T @ ns) @ feat_w^T + feat_b
  new_adj      = relu(A_aug @ (g_aug^T @ adj @ g_aug) @ A_aug^T)

The device streams adj (16 MB/batch, the only big input) once and contracts it
to Ga = g_aug^T @ adj  [9, N]; it also computes g_aug from ns and
PD = g_aug^T @ ns  [9, D].  The tiny rank-9 expansions to the [M, M] / [M, DO]
outputs (a few MFLOPs) are finished on the host, which keeps the device kernel
a pure stream: ~36 MB of DMA at the ~420 GB/s per-core HBM ceiling with a
minimal tail.

Device layout: row n of ns/adj lives at partition p = n // 16, sub-slot
c = n % 16  ("(p c)" order) so every DMA descriptor is a 16 KB contiguous run.
adj DMAs alternate between the two hardware DGE queues (sync + scalar engines)
so a blocked/refilling ring on one never starves the DMA engines.

If the zero-structure assumption ever fails, kernel() falls back to a faithful
numpy implementation of the reference.
"""

import numpy as np
from contextlib import ExitStack

B, N, D = 16, 2048, 256
H, M, DO = 8, 256, 256
NCORES = 8
BPC = B // NCORES          # batches per core
CH = 16                    # row sub-slots per partition (n = 16p + c)
CP = CH // 2               # chunk-pairs per batch for the adj stream
K9 = H + 1                 # augmented rank

_CACHE = {}


# --------------------------------------------------------------------------
# Tile tail workaround
# --------------------------------------------------------------------------
def _patch_tile_tail():
    """The stock Tile kernel tail (one Drain carrying every global-clock wait +
    EVSEM butterfly barriers) does not encode on this walrus build ("Too many
    sync wait commands" / "ISA wrong length").  Replace it with one-wait-per-
    Drain quiesce on the sync engine; semaphore cleanup is left to NRT's
    per-execution sema_reset preamble."""
    import concourse.tile as tile
    from concourse.vector_clock import ScopedClock, VectorClock

    if getattr(tile.TileContext, "_tail_patched", False):
        return

    def _drain_and_barrier(self, tick_clock, wait_clock):
        nc = self.nc
        gc = tick_clock.global_clock
        for p in range(len(gc)):
            t = gc[p]
            if t > 0:
                vc = VectorClock()
                vc.require_at_least(p, t)
                di = nc.sync.drain()
                wait_clock.add_sem_waits(di.ins, ScopedClock({None: vc}))
        popped = nc._tile_sem_poison_stack.pop()
        assert popped is self._sem_poison

    tile.TileContext._drain_and_barrier = _drain_and_barrier
    tile.TileContext._tail_patched = True


def _split_multi_waits(nc):
    """This walrus build encodes at most one sync-wait per instruction.  Tile's
    wait-assignment attaches several (e.g. a matmul waiting on its lhsT copy
    and its rhs DMA).  Hoist all but one wait onto NoOp instructions inserted
    immediately before, on the same engine — same-engine in-order dispatch
    preserves the blocking semantics exactly."""
    import concourse.mybir as mybir

    n_split = 0
    for fn in nc.m.functions:
        for blk in fn.blocks:
            insts = list(blk.instructions)
            out = []
            for inst in insts:
                si = getattr(inst, "sync_info", None)
                if si is not None and si.on_wait and len(si.on_wait) > 1:
                    waits = list(si.on_wait)
                    for w in waits[:-1]:
                        out.append(
                            mybir.InstNoOp(
                                name=f"waitsplit-{nc.next_id()}",
                                engine=inst.engine,
                                sync_info=mybir.SyncInfo(
                                    on_wait=[w], on_update=[]
                                ),
                                bass_nofuse=True,
                            )
                        )
                    inst.sync_info = mybir.SyncInfo(
                        on_wait=[waits[-1]], on_update=list(si.on_update)
                    )
                    n_split += 1
                out.append(inst)
            if len(out) != len(insts):
                blk.instructions = out
    return n_split


# --------------------------------------------------------------------------
# device kernel builder
# --------------------------------------------------------------------------
def _build_nc():
    import concourse.bass as bass
    import concourse.mybir as mybir
    import concourse.tile as tile

    _patch_tile_tail()

    FP = mybir.dt.float32
    BF = mybir.dt.bfloat16
    AF = mybir.ActivationFunctionType
    AXX = mybir.AxisListType.X

    nc = bass.Bass()
    p_ns = nc.declare_dram_parameter("node_set", [BPC, N, D], FP, isOutput=False)
    p_adj = nc.declare_dram_parameter("adj", [BPC, N, N], FP, isOutput=False)
    p_srep = nc.declare_dram_parameter("s_rep", [128, CH * K9], FP, isOutput=False)
    p_pd = nc.declare_dram_parameter("pd", [BPC, K9, D], FP, isOutput=True)
    p_g = nc.declare_dram_parameter("g", [BPC, 128, CH * K9], FP, isOutput=True)
    p_ga = nc.declare_dram_parameter("ga", [BPC, K9, N], FP, isOutput=True)

    with tile.TileContext(nc) as tc, ExitStack() as ctx:
        consts = ctx.enter_context(tc.tile_pool(name="consts", bufs=1))
        ns_pool = ctx.enter_context(tc.tile_pool(name="ns", bufs=2))
        nsbf_pool = ctx.enter_context(tc.tile_pool(name="nsbf", bufs=2))
        adj_pool = ctx.enter_context(tc.tile_pool(name="adj", bufs=6))
        abf_pool = ctx.enter_context(tc.tile_pool(name="adjbf", bufs=3))
        small = ctx.enter_context(tc.tile_pool(name="small", bufs=2))
        gasb_pool = ctx.enter_context(tc.tile_pool(name="gasb", bufs=2))
        ps_ga = ctx.enter_context(tc.tile_pool(name="psga", bufs=1, space="PSUM"))
        ps_pd = ctx.enter_context(tc.tile_pool(name="pspd", bufs=2, space="PSUM"))

        # ---- constants (DMA'd in the emission section, scalar ring head) ----
        s_rep = consts.tile([128, CH, K9], FP)

        # DRAM views in "(p c)" order: row n = 16p + c
        adj_pc = [p_adj[b].rearrange("(p c) j -> p c j", p=128) for b in range(BPC)]
        ns_pc = [p_ns[b].rearrange("(p c) d -> p c d", p=128) for b in range(BPC)]

        # ---- batch state ----
        st = [dict(), dict()]

        def ns_dma(b, eng):
            ns_sb = ns_pool.tile([128, CH, D], FP, tag="ns")
            eng.dma_start(ns_sb[:], ns_pc[b])
            st[b]["ns_sb"] = ns_sb

        def adj_dma(b, cp, eng):
            t = adj_pool.tile([128, 2, N], FP, tag="adj")
            eng.dma_start(t[:], adj_pc[b][:, 2 * cp : 2 * cp + 2, :])
            st[b][f"adj{cp}"] = t

        def g_phase(b):
            # g = rs / (S*rs + 1.6e-9), entirely on DVE (see module docstring)
            rs = small.tile([128, CH, 1], FP, tag="rs")
            nc.vector.reduce_sum(rs[:], st[b]["ns_sb"][:], axis=AXX)
            rs_b = rs[:].broadcast_to([128, CH, K9])
            den = small.tile([128, CH, K9], FP, tag="den")
            nc.vector.tensor_mul(den[:], rs_b, s_rep[:])
            nc.vector.tensor_scalar_add(den[:], den[:], 1.6e-9)
            gin = small.tile([128, CH, K9], FP, tag="gin")
            nc.vector.reciprocal(gin[:], den[:])
            g = small.tile([128, CH, K9], FP, tag="g")
            nc.vector.tensor_mul(g[:], rs_b, gin[:])
            nc.vector.memset(g[:, :, H : H + 1], 1.0)
            g_bf = small.tile([128, CH, K9], BF, tag="gbf")
            nc.vector.tensor_copy(g_bf[:], g[:])
            ns_bf = nsbf_pool.tile([128, CH, D], BF, tag="nsbf")
            nc.vector.tensor_copy(ns_bf[:], st[b]["ns_sb"][:])
            st[b]["g"], st[b]["g_bf"], st[b]["ns_bf"] = g, g_bf, ns_bf

        def cast_pair(b, cp):
            a = abf_pool.tile([128, 2, N], BF, tag="abf")
            nc.vector.tensor_copy(a[:], st[b][f"adj{cp}"][:])
            st[b][f"abf{cp}"] = a

        def mm_c(b, cp, k, stop_at=None):
            # Ga[:, j] += g[:, c]^T @ adj_rows(c)[:, j], j in 4 PSUM banks
            c = 2 * cp + k
            ga = st[b]["ga_ps"]
            gbf = st[b]["g_bf"]
            abf = st[b][f"abf{cp}"]
            for j4 in range(4):
                nc.tensor.matmul(
                    ga[:, j4 * 512 : (j4 + 1) * 512],
                    gbf[:, c, :],
                    abf[:, k, j4 * 512 : (j4 + 1) * 512],
                    start=(c == 0),
                    stop=(stop_at is not None and c == stop_at),
                )

        def pd_phase(b):
            pd_ps = ps_pd.tile([K9, D], FP, tag="pd_ps")
            for c in range(CH):
                nc.tensor.matmul(
                    pd_ps[:], st[b]["g_bf"][:, c, :], st[b]["ns_bf"][:, c, :],
                    start=(c == 0), stop=(c == CH - 1),
                )
            pd_sb = small.tile([K9, D], FP, tag="pd_sb")
            nc.vector.tensor_copy(pd_sb[:], pd_ps[:])
            st[b]["pd_sb"] = pd_sb

        # ==================================================================
        # emission — per-engine program order matches expected readiness.
        # sync ring:   ns0, b0 even pairs, b1 even pairs, ga1 outs   (~18 MB)
        # scalar ring: s_rep, ns1, b0 odd pairs, b1 odd pairs, tail
        #              pieces, small outs                            (~18 MB)
        # ==================================================================
        ga0_ps = ps_ga.tile([K9, N], FP, tag="ga")
        st[0]["ga_ps"] = ga0_ps

        ns_dma(0, nc.sync)
        nc.scalar.dma_start(s_rep[:], p_srep[:].rearrange("p (c k) -> p c k", k=K9))
        ns_dma(1, nc.scalar)
        for cp in range(CP):
            adj_dma(0, cp, nc.sync if cp % 2 == 0 else nc.scalar)
        g_phase(0)
        g_phase(1)

        cast_pair(0, 0)
        mm_c(0, 0, 0)
        mm_c(0, 0, 1)
        pd_phase(0)
        pd_phase(1)
        cast_pair(0, 1)
        mm_c(0, 1, 0)
        mm_c(0, 1, 1)
        for cp in range(2, CP):
            cast_pair(0, cp)
            mm_c(0, cp, 0)
            mm_c(0, cp, 1, stop_at=CH - 1)

        # batch-1 triggers (gated by adj tile frees)
        for cp in range(0, 7):
            adj_dma(1, cp, nc.sync if cp % 2 == 0 else nc.scalar)
        t14 = adj_pool.tile([128, 1, N], FP, tag="adj")
        nc.scalar.dma_start(t14[:], adj_pc[1][:, 14:15, :])
        t15a = adj_pool.tile([128, 1, N], FP, tag="adj")
        nc.scalar.dma_start(t15a[:, 0, 0:1024], adj_pc[1][:, 15:16, 0:1024])
        t15b = adj_pool.tile([128, 1, N], FP, tag="adj")
        nc.scalar.dma_start(t15b[:, 0, 1024:2048], adj_pc[1][:, 15:16, 1024:2048])

        # batch-0 Ga readout: 2 slices DVE + 2 gpsimd, out on the scalar ring
        ga0_sb = gasb_pool.tile([K9, N], FP, tag="ga_sb")
        nc.vector.tensor_copy(ga0_sb[:, 0:512], ga0_ps[:, 0:512])
        nc.vector.tensor_copy(ga0_sb[:, 512:1024], ga0_ps[:, 512:1024])
        nc.vector.tensor_copy(ga0_sb[:, 1024:1536], ga0_ps[:, 1024:1536])
        nc.vector.tensor_copy(ga0_sb[:, 1536:2048], ga0_ps[:, 1536:2048])

        ga1_ps = ps_ga.tile([K9, N], FP, tag="ga")
        st[1]["ga_ps"] = ga1_ps
        for cp in range(0, 7):
            cast_pair(1, cp)
            mm_c(1, cp, 0)
            mm_c(1, cp, 1)
        # tail: c14 cast split DVE/scalar, c15 halves on DVE + scalar
        a14 = abf_pool.tile([128, 1, N], BF, tag="abf")
        nc.vector.tensor_copy(a14[:, 0, 0:1024], t14[:, 0, 0:1024])
        nc.scalar.copy(a14[:, 0, 1024:2048], t14[:, 0, 1024:2048])
        g1bf = st[1]["g_bf"]
        for j4 in range(4):
            nc.tensor.matmul(
                ga1_ps[:, j4 * 512 : (j4 + 1) * 512],
                g1bf[:, 14, :],
                a14[:, 0, j4 * 512 : (j4 + 1) * 512],
                start=False, stop=False,
            )
        a15a = abf_pool.tile([128, 1, N], BF, tag="abf")
        nc.vector.tensor_copy(a15a[:, 0, 0:1024], t15a[:, 0, 0:1024])
        a15b = abf_pool.tile([128, 1, N], BF, tag="abf")
        nc.scalar.copy(a15b[:, 0, 1024:2048], t15b[:, 0, 1024:2048])
        for j4 in range(2):
            nc.tensor.matmul(
                ga1_ps[:, j4 * 512 : (j4 + 1) * 512],
                g1bf[:, 15, :],
                a15a[:, 0, j4 * 512 : (j4 + 1) * 512],
                start=False, stop=True,
            )
        for j4 in range(2, 4):
            nc.tensor.matmul(
                ga1_ps[:, j4 * 512 : (j4 + 1) * 512],
                g1bf[:, 15, :],
                a15b[:, 0, j4 * 512 : (j4 + 1) * 512],
                start=False, stop=True,
            )
        ga1_sb = gasb_pool.tile([K9, N], FP, tag="ga_sb")
        nc.vector.tensor_copy(ga1_sb[:, 0:512], ga1_ps[:, 0:512])
        nc.vector.tensor_copy(ga1_sb[:, 512:1024], ga1_ps[:, 512:1024])
        nc.sync.dma_start(p_ga[1, :, 0:1024], ga1_sb[:, 0:1024])
        nc.scalar.copy(ga1_sb[:, 1024:1536], ga1_ps[:, 1024:1536])
        nc.scalar.copy(ga1_sb[:, 1536:2048], ga1_ps[:, 1536:2048])
        nc.sync.dma_start(p_ga[1, :, 1024:2048], ga1_sb[:, 1024:2048])

        # small outputs — scalar ring tail (triggers fire as data readies)
        nc.scalar.dma_start(p_pd[0], st[0]["pd_sb"][:])
        nc.scalar.dma_start(
            p_g[0].rearrange("p (c k) -> p c k", k=K9), st[0]["g"][:]
        )
        nc.scalar.dma_start(p_ga[0], ga0_sb[:])
        nc.scalar.dma_start(p_pd[1], st[1]["pd_sb"][:])
        nc.scalar.dma_start(
            p_g[1].rearrange("p (c k) -> p c k", k=K9), st[1]["g"][:]
        )

    _split_multi_waits(nc)
    return nc


# --------------------------------------------------------------------------
# host-side parameter folding + rank-9 output expansion
# --------------------------------------------------------------------------
def _prep_consts(i2c_w, i2c_b, lin_b, conv_w, conv_b, feat_w, feat_b):
    if not (np.all(i2c_w == 0.0) and np.all(i2c_b == 0.0)):
        return None
    r = np.maximum(np.asarray(lin_b, np.float32), 0.0).reshape(H, M)
    cval = r / np.maximum(np.float32(np.sqrt(D)) * r, np.float32(1e-6))  # [H,M]
    S = cval.sum(axis=1, dtype=np.float32)                                # [H]
    A = (np.asarray(conv_w, np.float32)[:, None] * cval).T                # [M,H]
    A_aug = np.concatenate(
        [A, np.full((M, 1), np.float32(conv_b[0]), np.float32)], axis=1
    )                                                                     # [M,9]
    s_rep = np.tile(
        np.concatenate([S, np.zeros(1, np.float32)]), CH
    )[None, :].repeat(128, axis=0)                                        # [128,144]
    return {
        "s_rep": np.ascontiguousarray(s_rep),
        "host_a_aug": A_aug,
        "host_featw": np.asarray(feat_w, np.float32),
        "host_featb": np.asarray(feat_b, np.float32),
    }


def _run_device(node_set, adj, consts, trace=False):
    from concourse.bass_utils import run_bass_kernel_spmd

    if "nc" not in _CACHE:
        _CACHE["nc"] = _build_nc()
    nc = _CACHE["nc"]
    dev_consts = {"s_rep": consts["s_rep"]}
    in_maps = []
    for i in range(NCORES):
        in_maps.append(
            {
                "node_set": np.ascontiguousarray(node_set[i * BPC : (i + 1) * BPC]),
                "adj": np.ascontiguousarray(adj[i * BPC : (i + 1) * BPC]),
                **dev_consts,
            }
        )
    res = run_bass_kernel_spmd(
        nc, in_maps, core_ids=list(range(NCORES)), trace=trace
    )
    pd = np.concatenate([r["pd"] for r in res.results], axis=0)   # [B,9,D]
    gm = np.concatenate([r["g"] for r in res.results], axis=0)    # [B,128,144]
    ga = np.concatenate([r["ga"] for r in res.results], axis=0)   # [B,9,N]

    A_aug = consts["host_a_aug"]                                  # [M,9]
    featw = consts["host_featw"]                                  # [DO,D]
    featb = consts["host_featb"]                                  # [DO]
    # g device layout: partition p, slot c, k  ->  row n = 16p + c
    g_full = gm.reshape(B, N, K9)                                 # [B,N,9]
    W = np.einsum("bkn,bnl->bkl", ga, g_full).astype(np.float32)  # [B,9,9]
    out2 = np.maximum(
        np.einsum("mk,bkl,ol->bmo", A_aug, W, A_aug), 0.0
    ).astype(np.float32)                                          # [B,M,M]
    out1 = (
        np.einsum("mk,bkd,od->bmo", A_aug, pd, featw) + featb[None, None, :]
    ).astype(np.float32)                                          # [B,M,DO]
    return (out1, out2), res


# --------------------------------------------------------------------------
# numpy fallback (faithful port of the jax reference; not expected to run)
# --------------------------------------------------------------------------
def _reference_numpy(node_set, adj, W_0, i2c_w, i2c_b, lin_w, lin_b,
                     conv_w, conv_b, feat_w, feat_b):
    f32 = np.float32
    ns = np.asarray(node_set, f32)
    b = ns.shape[0]
    temp = ns.mean(axis=1, keepdims=True)
    h_avg = np.tanh(temp @ np.asarray(W_0, f32))
    att = np.einsum("bnd,bod->bno", ns, h_avg).astype(f32)
    bc = np.einsum("bno,bnd->bod", att, ns).astype(f32)
    x = np.transpose(bc, (0, 2, 1))
    x = np.maximum(x @ np.asarray(i2c_w, f32).T + np.asarray(i2c_b, f32), 0)
    x = np.maximum(x @ np.asarray(lin_w, f32).T + np.asarray(lin_b, f32), 0)
    centroids = np.transpose(x, (0, 2, 1)).reshape(b, H, M, D)
    ns_n = ns / np.maximum(
        np.linalg.norm(ns, axis=-1, keepdims=True), 1e-6
    ).astype(f32)
    c_n = centroids / np.maximum(
        np.linalg.norm(centroids, axis=-1, keepdims=True), 1e-6
    ).astype(f32)
    C_heads = np.einsum("bhmd,bnd->bhmn", c_n, ns_n).astype(f32)
    normalizer = C_heads.sum(axis=2, keepdims=True)
    C_heads = C_heads / (normalizer + f32(1e-10))
    C = np.einsum("bhmn,h->bmn", C_heads, np.asarray(conv_w, f32)).astype(f32) \
        + f32(conv_b[0])
    nns = (C @ ns) @ np.asarray(feat_w, f32).T + np.asarray(feat_b, f32)
    q_adj = C @ np.asarray(adj, f32)
    new_adj = np.maximum(q_adj @ np.transpose(C, (0, 2, 1)), 0)
    return nns.astype(f32), new_adj.astype(f32)


# --------------------------------------------------------------------------
# entry point
# --------------------------------------------------------------------------
def kernel(node_set, adj, W_0, i2c_w, i2c_b, lin_w, lin_b, conv_w, conv_b,
           feat_w, feat_b):
    consts = _prep_consts(i2c_w, i2c_b, lin_b, conv_w, conv_b, feat_w, feat_b)
    if consts is None:
        return _reference_numpy(node_set, adj, W_0, i2c_w, i2c_b, lin_w, lin_b,
                                conv_w, conv_b, feat_w, feat_b)
    (out1, out2), _ = _run_device(
        np.ascontiguousarray(np.asarray(node_set, np.float32)),
        np.ascontiguousarray(np.asarray(adj, np.float32)),
        consts,
    )
    return out1, out2


# revision 11
# speedup vs baseline: 1.0215x; 1.0215x over previous
"""Trainium2 Bass kernel for nn_Adaptive_Pooling_Layer (B=16, N=2048, D=256, H=8, M=256).

Data-parallel over batch: 8 NeuronCores x 2 batches each; params replicated.

Math notes
----------
The module's input2centroids layer has weight/bias == 0 (torch zeros init), so
x = relu(bc @ 0 + 0) = 0 and centroids = relu(lin_b) broadcast over (batch, d):
centroids[b,h,m,d] = r[h,m] := relu(lin_b[h*M+m])  (constant in b and d).
Hence c_n[h,m,d] = cval[h,m] := r / max(16*r, 1e-6)   (sqrt(D)=16), and with
  rs[n]  = sum_d ns[n,d],   S[h] = sum_m cval[h,m]
  g[n,h] = t/(S[h]*t + 1e-10)  with t = rs/||ns[n,:]||
         = rs[n] / (S[h]*rs[n] + 1e-10*||ns[n,:]||)
        ~= rs[n] / (S[h]*rs[n] + 1.6e-9)          (||row|| = 16 +- 0.7 whp;
  the 1e-10 guard only matters at |rs| ~ 1e-9 which randn inputs never hit,
  so the row norm - and with it the whole sqrt/square pipeline - drops out)
the normalized C_heads[b,h,m,n] = cval[h,m] * g[b,n,h], so C = A_aug @ g_aug^T
with A_aug = [conv_w*cval^T | conv_b*1]  (M x 9).  Then
  new_node_set = A_aug @ (g_aug^T @ ns) @ feat_w^T + feat_b
  new_adj      = relu(A_aug @ (g_aug^T @ adj @ g_aug) @ A_aug^T)

The device streams adj (16 MB/batch, the only big input) once and contracts it
to Ga = g_aug^T @ adj  [9, N]; it also computes g_aug from ns and
PD = g_aug^T @ ns  [9, D].  The tiny rank-9 expansions to the [M, M] / [M, DO]
outputs (a few MFLOPs) are finished on the host, which keeps the device kernel
a pure stream: ~36 MB of DMA at the ~420 GB/s per-core HBM ceiling with a
minimal tail.

Device layout: row n of ns/adj lives at partition p = n // 16, sub-slot
c = n % 16  ("(p c)" order) so every DMA descriptor is a 16 KB contiguous run.
adj DMAs alternate between the two hardware DGE queues (sync + scalar engines)
so a blocked/refilling ring on one never starves the DMA engines.

If the zero-structure assumption ever fails, kernel() falls back to a faithful
numpy implementation of the reference.
"""

import numpy as np
from contextlib import ExitStack

B, N, D = 16, 2048, 256
H, M, DO = 8, 256, 256
NCORES = 8
BPC = B // NCORES          # batches per core
CH = 16                    # row sub-slots per partition (n = 16p + c)
CP = CH // 2               # chunk-pairs per batch for the adj stream
K9 = H + 1                 # augmented rank

_CACHE = {}


# --------------------------------------------------------------------------
# Tile tail workaround
# --------------------------------------------------------------------------
def _patch_tile_tail():
    """The stock Tile kernel tail (one Drain carrying every global-clock wait +
    EVSEM butterfly barriers) does not encode on this walrus build ("Too many
    sync wait commands" / "ISA wrong length").  Replace it with one-wait-per-
    Drain quiesce on the sync engine; semaphore cleanup is left to NRT's
    per-execution sema_reset preamble."""
    import concourse.tile as tile
    from concourse.vector_clock import ScopedClock, VectorClock

    if getattr(tile.TileContext, "_tail_patched", False):
        return

    def _drain_and_barrier(self, tick_clock, wait_clock):
        nc = self.nc
        gc = tick_clock.global_clock
        for p in range(len(gc)):
            t = gc[p]
            if t > 0:
                vc = VectorClock()
                vc.require_at_least(p, t)
                di = nc.sync.drain()
                wait_clock.add_sem_waits(di.ins, ScopedClock({None: vc}))
        popped = nc._tile_sem_poison_stack.pop()
        assert popped is self._sem_poison

    tile.TileContext._drain_and_barrier = _drain_and_barrier
    tile.TileContext._tail_patched = True


def _split_multi_waits(nc):
    """This walrus build encodes at most one sync-wait per instruction.  Tile's
    wait-assignment attaches several (e.g. a matmul waiting on its lhsT copy
    and its rhs DMA).  Hoist all but one wait onto NoOp instructions inserted
    immediately before, on the same engine — same-engine in-order dispatch
    preserves the blocking semantics exactly."""
    import concourse.mybir as mybir

    n_split = 0
    for fn in nc.m.functions:
        for blk in fn.blocks:
            insts = list(blk.instructions)
            out = []
            for inst in insts:
                si = getattr(inst, "sync_info", None)
                if si is not None and si.on_wait and len(si.on_wait) > 1:
                    waits = list(si.on_wait)
                    for w in waits[:-1]:
                        out.append(
                            mybir.InstNoOp(
                                name=f"waitsplit-{nc.next_id()}",
                                engine=inst.engine,
                                sync_info=mybir.SyncInfo(
                                    on_wait=[w], on_update=[]
                                ),
                                bass_nofuse=True,
                            )
                        )
                    inst.sync_info = mybir.SyncInfo(
                        on_wait=[waits[-1]], on_update=list(si.on_update)
                    )
                    n_split += 1
                out.append(inst)
            if len(out) != len(insts):
                blk.instructions = out
    return n_split


# --------------------------------------------------------------------------
# device kernel builder
# --------------------------------------------------------------------------
def _build_nc():
    import concourse.bass as bass
    import concourse.mybir as mybir
    import concourse.tile as tile

    _patch_tile_tail()

    FP = mybir.dt.float32
    BF = mybir.dt.bfloat16
    AXX = mybir.AxisListType.X

    nc = bass.Bass()
    p_ns = nc.declare_dram_parameter("node_set", [BPC, N, D], FP, isOutput=False)
    p_adj = nc.declare_dram_parameter("adj", [BPC, N, N], FP, isOutput=False)
    p_srep = nc.declare_dram_parameter("s_rep", [128, BPC * CH * K9], FP,
                                       isOutput=False)
    p_pd = nc.declare_dram_parameter("pd", [BPC, K9, D], FP, isOutput=True)
    p_g = nc.declare_dram_parameter("g", [128, BPC * CH * K9], FP, isOutput=True)
    p_ga = nc.declare_dram_parameter("ga", [BPC, K9, N], FP, isOutput=True)

    with tile.TileContext(nc) as tc, ExitStack() as ctx:
        consts = ctx.enter_context(tc.tile_pool(name="consts", bufs=1))
        ns_pool = ctx.enter_context(tc.tile_pool(name="ns", bufs=2))
        nsbf_pool = ctx.enter_context(tc.tile_pool(name="nsbf", bufs=2))
        adj_pool = ctx.enter_context(tc.tile_pool(name="adj", bufs=6))
        abf_pool = ctx.enter_context(tc.tile_pool(name="adjbf", bufs=4))
        small = ctx.enter_context(tc.tile_pool(name="small", bufs=1))
        gasb_pool = ctx.enter_context(tc.tile_pool(name="gasb", bufs=2))
        ps_ga = ctx.enter_context(tc.tile_pool(name="psga", bufs=1, space="PSUM"))
        ps_pd = ctx.enter_context(tc.tile_pool(name="pspd", bufs=2, space="PSUM"))

        s_rep = consts.tile([128, BPC, CH, K9], FP)

        # DRAM views in "(p c)" order: row n = 16p + c
        adj_pc = [p_adj[b].rearrange("(p c) j -> p c j", p=128) for b in range(BPC)]
        ns_pc = [p_ns[b].rearrange("(p c) d -> p c d", p=128) for b in range(BPC)]

        st = [dict(), dict()]

        def adj_dma(b, cp, eng):
            t = adj_pool.tile([128, 2, N], FP, tag="adj")
            eng.dma_start(t[:], adj_pc[b][:, 2 * cp : 2 * cp + 2, :])
            st[b][f"adj{cp}"] = t

        # g = rs / (S*rs + 1.6e-9) on DVE (see module docstring); both batches
        # share one tile, written in c-slices so the PE can start early.
        rs = consts.tile([128, BPC, CH, 1], FP)
        den = consts.tile([128, BPC, CH, K9], FP)
        gin = consts.tile([128, BPC, CH, K9], FP)
        g = consts.tile([128, BPC, CH, K9], FP)
        g_bf = consts.tile([128, BPC, CH, K9], BF)

        def g_slice(b, c0, c1):
            sl = (slice(None), b, slice(c0, c1))
            nc.vector.reduce_sum(rs[sl], st[b]["ns_sb"][:, c0:c1, :], axis=AXX)
            rs_b = rs[sl].broadcast_to([128, c1 - c0, K9])
            nc.vector.tensor_mul(den[sl], rs_b, s_rep[sl])
            nc.vector.tensor_scalar_add(den[sl], den[sl], 1.6e-9)
            nc.vector.reciprocal(gin[sl], den[sl])
            nc.vector.tensor_mul(g[sl], rs_b, gin[sl])
            nc.vector.memset(g[:, b, c0:c1, H : H + 1], 1.0)
            nc.vector.tensor_copy(g_bf[sl], g[sl])

        def cast_pair(b, cp):
            a = abf_pool.tile([128, 2, N], BF, tag="abf")
            nc.vector.tensor_copy(a[:], st[b][f"adj{cp}"][:])
            st[b][f"abf{cp}"] = a

        def mm_c(b, cp, k, stop_at=None):
            c = 2 * cp + k
            ga = st[b]["ga_ps"]
            abf = st[b][f"abf{cp}"]
            for j4 in range(4):
                nc.tensor.matmul(
                    ga[:, j4 * 512 : (j4 + 1) * 512],
                    g_bf[:, b, c, :],
                    abf[:, k, j4 * 512 : (j4 + 1) * 512],
                    start=(c == 0),
                    stop=(stop_at is not None and c == stop_at),
                )

        def pd_phase(b):
            ns_bf = nsbf_pool.tile([128, CH, D], BF, tag="nsbf")
            nc.vector.tensor_copy(ns_bf[:], st[b]["ns_sb"][:])
            pd_ps = ps_pd.tile([K9, D], FP, tag="pd_ps")
            for c in range(CH):
                nc.tensor.matmul(
                    pd_ps[:], g_bf[:, b, c, :], ns_bf[:, c, :],
                    start=(c == 0), stop=(c == CH - 1),
                )
            pd_sb = small.tile([K9, D], FP, tag=f"pd_sb{b}")
            nc.vector.tensor_copy(pd_sb[:], pd_ps[:])
            st[b]["pd_sb"] = pd_sb

        # ==================================================================
        # emission — per-engine program order matches expected readiness.
        # sync ring:   ns0, b0 evens, b1 cp0/2/4, t14, t15b   (~17.5 MB)
        # scalar ring: s_rep, ns1, b0 odds, b1 cp1/3/5/6, t15a, outs (~18.6 MB)
        # ==================================================================
        ga0_ps = ps_ga.tile([K9, N], FP, tag="ga")
        st[0]["ga_ps"] = ga0_ps

        ns0 = ns_pool.tile([128, CH, D], FP, tag="ns")
        nc.sync.dma_start(ns0[:], ns_pc[0])
        st[0]["ns_sb"] = ns0
        nc.scalar.dma_start(
            s_rep[:], p_srep[:].rearrange("p (b c k) -> p b c k", b=BPC, k=K9)
        )
        ns1 = ns_pool.tile([128, CH, D], FP, tag="ns")
        nc.scalar.dma_start(ns1[:], ns_pc[1])
        st[1]["ns_sb"] = ns1
        for cp in range(CP):
            adj_dma(0, cp, nc.sync if cp % 2 == 0 else nc.scalar)

        g_slice(0, 0, 4)
        cast_pair(0, 0)
        mm_c(0, 0, 0)
        mm_c(0, 0, 1)
        g_slice(0, 4, CH)
        cast_pair(0, 1)
        mm_c(0, 1, 0)
        mm_c(0, 1, 1)
        g_slice(1, 0, CH)
        cast_pair(0, 2)
        mm_c(0, 2, 0)
        mm_c(0, 2, 1)
        pd_phase(0)
        cast_pair(0, 3)
        mm_c(0, 3, 0)
        mm_c(0, 3, 1)
        pd_phase(1)
        for cp in range(4, CP):
            cast_pair(0, cp)
            mm_c(0, cp, 0)
            mm_c(0, cp, 1, stop_at=CH - 1)

        # batch-1 triggers (gated by adj tile frees / ring backpressure)
        adj_dma(1, 0, nc.sync)
        adj_dma(1, 1, nc.scalar)
        adj_dma(1, 2, nc.sync)
        adj_dma(1, 3, nc.scalar)
        adj_dma(1, 4, nc.sync)
        adj_dma(1, 5, nc.scalar)
        adj_dma(1, 6, nc.scalar)
        t14 = adj_pool.tile([128, 1, N], FP, tag="adj")
        nc.sync.dma_start(t14[:], adj_pc[1][:, 14:15, :])
        t15a = adj_pool.tile([128, 1, N], FP, tag="adj")
        nc.scalar.dma_start(t15a[:, 0, 0:1024], adj_pc[1][:, 15:16, 0:1024])
        t15b = adj_pool.tile([128, 1, N], FP, tag="adj")
        nc.sync.dma_start(t15b[:, 0, 1024:2048], adj_pc[1][:, 15:16, 1024:2048])

        # batch-0 Ga readout: 2 slices DVE + 2 scalar, out on the scalar ring
        ga0_sb = gasb_pool.tile([K9, N], FP, tag="ga_sb")
        nc.vector.tensor_copy(ga0_sb[:, 0:512], ga0_ps[:, 0:512])
        nc.vector.tensor_copy(ga0_sb[:, 512:1024], ga0_ps[:, 512:1024])
        nc.scalar.copy(ga0_sb[:, 1024:1536], ga0_ps[:, 1024:1536])
        nc.scalar.copy(ga0_sb[:, 1536:2048], ga0_ps[:, 1536:2048])

        ga1_ps = ps_ga.tile([K9, N], FP, tag="ga")
        st[1]["ga_ps"] = ga1_ps
        for cp in range(0, 7):
            cast_pair(1, cp)
            mm_c(1, cp, 0)
            mm_c(1, cp, 1)
        # tail: c14 cast split DVE/scalar, c15 halves on DVE + scalar
        a14 = abf_pool.tile([128, 1, N], BF, tag="abf")
        nc.vector.tensor_copy(a14[:, 0, 0:1024], t14[:, 0, 0:1024])
        nc.scalar.copy(a14[:, 0, 1024:2048], t14[:, 0, 1024:2048])
        for j4 in range(4):
            nc.tensor.matmul(
                ga1_ps[:, j4 * 512 : (j4 + 1) * 512],
                g_bf[:, 1, 14, :],
                a14[:, 0, j4 * 512 : (j4 + 1) * 512],
                start=False, stop=False,
            )
        a15a = abf_pool.tile([128, 1, N], BF, tag="abf")
        nc.vector.tensor_copy(a15a[:, 0, 0:1024], t15a[:, 0, 0:1024])
        a15b = abf_pool.tile([128, 1, N], BF, tag="abf")
        nc.scalar.copy(a15b[:, 0, 1024:2048], t15b[:, 0, 1024:2048])
        for j4 in range(2):
            nc.tensor.matmul(
                ga1_ps[:, j4 * 512 : (j4 + 1) * 512],
                g_bf[:, 1, 15, :],
                a15a[:, 0, j4 * 512 : (j4 + 1) * 512],
                start=False, stop=True,
            )
        for j4 in range(2, 4):
            nc.tensor.matmul(
                ga1_ps[:, j4 * 512 : (j4 + 1) * 512],
                g_bf[:, 1, 15, :],
                a15b[:, 0, j4 * 512 : (j4 + 1) * 512],
                start=False, stop=True,
            )
        ga1_sb = gasb_pool.tile([K9, N], FP, tag="ga_sb")
        nc.vector.tensor_copy(ga1_sb[:, 0:512], ga1_ps[:, 0:512])
        nc.vector.tensor_copy(ga1_sb[:, 512:1024], ga1_ps[:, 512:1024])
        nc.sync.dma_start(p_ga[1, :, 0:1024], ga1_sb[:, 0:1024])
        nc.scalar.copy(ga1_sb[:, 1024:1536], ga1_ps[:, 1024:1536])
        nc.scalar.copy(ga1_sb[:, 1536:2048], ga1_ps[:, 1536:2048])
        nc.sync.dma_start(p_ga[1, :, 1024:2048], ga1_sb[:, 1024:2048])

        # small outputs — scalar ring tail (triggers fire as data readies)
        nc.scalar.dma_start(
            p_g[:].rearrange("p (b c k) -> p b c k", b=BPC, k=K9), g[:]
        )
        nc.scalar.dma_start(p_pd[0], st[0]["pd_sb"][:])
        nc.scalar.dma_start(p_ga[0], ga0_sb[:])
        nc.scalar.dma_start(p_pd[1], st[1]["pd_sb"][:])

    _split_multi_waits(nc)
    return nc


# --------------------------------------------------------------------------
# host-side parameter folding + rank-9 output expansion
# --------------------------------------------------------------------------
def _prep_consts(i2c_w, i2c_b, lin_b, conv_w, conv_b, feat_w, feat_b):
    if not (np.all(i2c_w == 0.0) and np.all(i2c_b == 0.0)):
        return None
    r = np.maximum(np.asarray(lin_b, np.float32), 0.0).reshape(H, M)
    cval = r / np.maximum(np.float32(np.sqrt(D)) * r, np.float32(1e-6))  # [H,M]
    S = cval.sum(axis=1, dtype=np.float32)                                # [H]
    A = (np.asarray(conv_w, np.float32)[:, None] * cval).T                # [M,H]
    A_aug = np.concatenate(
        [A, np.full((M, 1), np.float32(conv_b[0]), np.float32)], axis=1
    )                                                                     # [M,9]
    s_rep = np.tile(
        np.concatenate([S, np.zeros(1, np.float32)]), BPC * CH
    )[None, :].repeat(128, axis=0)                                        # [128,288]
    return {
        "s_rep": np.ascontiguousarray(s_rep),
        "host_a_aug": A_aug,
        "host_featw": np.asarray(feat_w, np.float32),
        "host_featb": np.asarray(feat_b, np.float32),
    }


def _run_device(node_set, adj, consts, trace=False):
    from concourse.bass_utils import run_bass_kernel_spmd

    if "nc" not in _CACHE:
        _CACHE["nc"] = _build_nc()
    nc = _CACHE["nc"]
    dev_consts = {"s_rep": consts["s_rep"]}
    in_maps = []
    for i in range(NCORES):
        in_maps.append(
            {
                "node_set": np.ascontiguousarray(node_set[i * BPC : (i + 1) * BPC]),
                "adj": np.ascontiguousarray(adj[i * BPC : (i + 1) * BPC]),
                **dev_consts,
            }
        )
    res = run_bass_kernel_spmd(
        nc, in_maps, core_ids=list(range(NCORES)), trace=trace
    )
    pd = np.concatenate([r["pd"] for r in res.results], axis=0)   # [B,9,D]
    gm = np.stack([r["g"] for r in res.results], axis=0)          # [NC,128,288]
    ga = np.concatenate([r["ga"] for r in res.results], axis=0)   # [B,9,N]

    A_aug = consts["host_a_aug"]                                  # [M,9]
    featw = consts["host_featw"]                                  # [DO,D]
    featb = consts["host_featb"]                                  # [DO]
    # g device layout: partition p, batch b, slot c, k  ->  row n = 16p + c
    g_full = gm.reshape(NCORES, 128, BPC, CH, K9).transpose(0, 2, 1, 3, 4)
    g_full = np.ascontiguousarray(g_full).reshape(B, N, K9)       # [B,N,9]
    W = np.einsum("bkn,bnl->bkl", ga, g_full).astype(np.float32)  # [B,9,9]
    out2 = np.maximum(
        np.einsum("mk,bkl,ol->bmo", A_aug, W, A_aug), 0.0
    ).astype(np.float32)                                          # [B,M,M]
    out1 = (
        np.einsum("mk,bkd,od->bmo", A_aug, pd, featw) + featb[None, None, :]
    ).astype(np.float32)                                          # [B,M,DO]
    return (out1, out2), res


# --------------------------------------------------------------------------
# numpy fallback (faithful port of the jax reference; not expected to run)
# --------------------------------------------------------------------------
def _reference_numpy(node_set, adj, W_0, i2c_w, i2c_b, lin_w, lin_b,
                     conv_w, conv_b, feat_w, feat_b):
    f32 = np.float32
    ns = np.asarray(node_set, f32)
    b = ns.shape[0]
    temp = ns.mean(axis=1, keepdims=True)
    h_avg = np.tanh(temp @ np.asarray(W_0, f32))
    att = np.einsum("bnd,bod->bno", ns, h_avg).astype(f32)
    bc = np.einsum("bno,bnd->bod", att, ns).astype(f32)
    x = np.transpose(bc, (0, 2, 1))
    x = np.maximum(x @ np.asarray(i2c_w, f32).T + np.asarray(i2c_b, f32), 0)
    x = np.maximum(x @ np.asarray(lin_w, f32).T + np.asarray(lin_b, f32), 0)
    centroids = np.transpose(x, (0, 2, 1)).reshape(b, H, M, D)
    ns_n = ns / np.maximum(
        np.linalg.norm(ns, axis=-1, keepdims=True), 1e-6
    ).astype(f32)
    c_n = centroids / np.maximum(
        np.linalg.norm(centroids, axis=-1, keepdims=True), 1e-6
    ).astype(f32)
    C_heads = np.einsum("bhmd,bnd->bhmn", c_n, ns_n).astype(f32)
    normalizer = C_heads.sum(axis=2, keepdims=True)
    C_heads = C_heads / (normalizer + f32(1e-10))
    C = np.einsum("bhmn,h->bmn", C_heads, np.asarray(conv_w, f32)).astype(f32) \
        + f32(conv_b[0])
    nns = (C @ ns) @ np.asarray(feat_w, f32).T + np.asarray(feat_b, f32)
    q_adj = C @ np.asarray(adj, f32)
    new_adj = np.maximum(q_adj @ np.transpose(C, (0, 2, 1)), 0)
    return nns.astype(f32), new_adj.astype(f32)


# --------------------------------------------------------------------------
# entry point
# --------------------------------------------------------------------------
def kernel(node_set, adj, W_0, i2c_w, i2c_b, lin_w, lin_b, conv_w, conv_b,
           feat_w, feat_b):
    consts = _prep_consts(i2c_w, i2c_b, lin_b, conv_w, conv_b, feat_w, feat_b)
    if consts is None:
        return _reference_numpy(node_set, adj, W_0, i2c_w, i2c_b, lin_w, lin_b,
                                conv_w, conv_b, feat_w, feat_b)
    (out1, out2), _ = _run_device(
        np.ascontiguousarray(np.asarray(node_set, np.float32)),
        np.ascontiguousarray(np.asarray(adj, np.float32)),
        consts,
    )
    return out1, out2
